# revision 1
# baseline (speedup 1.0000x reference)
"""BFP (block floating point) quantizer for Trainium2, 8 NeuronCores.

Reference semantics (BITWIDTH=16, BLOCK_SIZE=16, AXIS=1):
  per 16-element block along axis 1:
    max_abs = max |x|                     (block reduction)
    shared_exp = frexp(max_abs).e - 1
    step = 2^(shared_exp - 6)
    q = clip(round_half_even(x / step), -127, 127) * step
    q = 0 where max_abs == 0

Kernel mapping (per [128, 8192] f32 tile, blocks of 16 on the free axis):
  1. m = tensor_reduce(max, abs) over [128, 512, 16]        -> block max-abs
  2. exponent bit tricks on the int32 view of m:
       masked     = m_bits & 0x7F800000          (sign is 0, m >= 0)
       step_bits  = max(masked, 7<<23) - 6<<23   (power of two, exact;
                                                  the max() guards all-zero /
                                                  denormal blocks)
       rstep_bits = 0x7F000000 - step_bits       (exact reciprocal: exponents
                                                  sum to 254)
  3. y_i8 = tensor_tensor(x, rstep_bcast, mult) with int8 output.
     x * rstep is exact in fp32 (power-of-two scale); the DVE output
     converter does RNE + saturation, which implements round-half-even and
     the upper clip in one pass (verified bit-exact on HW).  The only
     deviation from the reference is y in (-128, -127.5] -> -128 (reference
     clips to -127), fixed in step 4.
  4. q = scalar_tensor_tensor(y_i8, -127.0, step_bcast, max, mult) -> f32.

Sharding: trivially data-parallel on axis 0; each of the 8 cores gets a
[1024, 8192] row shard and runs 8 [128, 8192] tiles.
"""

import sys

for _p in ("/opt/trn_rl_repo",):
    if _p not in sys.path:
        sys.path.append(_p)

import json

import numpy as np

N_CORES = 8
R_FULL = 8192
C = 8192
R_LOCAL = R_FULL // N_CORES  # 1024
P = 128
BLK = 16
NB = C // BLK  # 512
N_TILES = R_LOCAL // P  # 8


# ---------------------------------------------------------------------------
# Workaround for this container's walrus build: it encodes at most ONE
# semaphore wait per instruction ("Too many sync wait commands").  Rewrite the
# serialized BIR so any instruction with N>1 waits is preceded by N-1
# same-engine NoOps carrying one wait each.
# ---------------------------------------------------------------------------
def _split_multiwaits(bir_json: bytes) -> bytes:
    j = json.loads(bir_json)
    ctr = 0
    changed = False
    for fn in j.get("functions", []):
        for bb in fn.get("blocks", []):
            new_insts = []
            for ins in bb.get("instructions", []):
                si = ins.get("sync_info")
                waits = (si or {}).get("on_wait") or []
                if len(waits) > 1:
                    changed = True
                    for w in waits[:-1]:
                        ctr += 1
                        carrier = {
                            "engine": ins["engine"],
                            "ins": [],
                            "outs": [],
                            "name": f"WSPLIT-{ctr}",
                            "opcode": "NoOp",
                            "text_hint": "wait_split",
                            "sync_info": {"on_wait": [w], "on_update": []},
                        }
                        if "debug" in ins:
                            carrier["debug"] = ins["debug"]
                        new_insts.append(carrier)
                    si["on_wait"] = [waits[-1]]
                new_insts.append(ins)
            bb["instructions"] = new_insts
    if not changed:
        return bir_json
    return json.dumps(j).encode()


_hook_applied = False


def _apply_bir_fix():
    global _hook_applied
    if _hook_applied:
        return
    _hook_applied = True
    from concourse import bass2jax

    orig = bass2jax.compile_bir_kernel

    def wrapper(bir_json, tmpdir, neff_name="file.neff"):
        return orig(_split_multiwaits(bytes(bir_json)), tmpdir, neff_name)

    bass2jax.compile_bir_kernel = wrapper


# ---------------------------------------------------------------------------
# Program construction
# ---------------------------------------------------------------------------
def build_program(reps: int = 1):
    """reps>1 wraps the whole tile loop in a dynamic For_i — used only for
    benchmarking (amortizes the ~80ms axon dispatch overhead)."""
    from contextlib import nullcontext

    import concourse.bass as bass
    import concourse.tile as tile
    from concourse import mybir

    F32 = mybir.dt.float32
    I32 = mybir.dt.int32
    I8 = mybir.dt.int8

    nc = bass.Bass("TRN2", target_bir_lowering=False)
    x_ext = nc.dram_tensor("x", [R_LOCAL, C], F32, kind="ExternalInput")
    out_ext = nc.dram_tensor("out", [R_LOCAL, C], F32, kind="ExternalOutput")

    with tile.TileContext(nc) as tc:
        with (
            tc.tile_pool(name="xin", bufs=2) as xin,
            tc.tile_pool(name="qout", bufs=2) as qout,
            tc.tile_pool(name="i8p", bufs=2) as i8p,
            tc.tile_pool(name="small", bufs=2) as small,
            tc.tile_pool(name="consts", bufs=1) as consts,
            tc.For_i(0, reps, 1) if reps > 1 else nullcontext(),
        ):
            # step_bits + rstep_bits = 254 << 23
            csum = consts.tile([P, 1], I32)
            nc.vector.memset(csum, 0x7F000000)

            for i in range(N_TILES):
                rows = slice(i * P, (i + 1) * P)

                x_t = xin.tile([P, C], F32)
                x3 = x_t.rearrange("p (b k) -> p b k", k=BLK)
                m = small.tile([P, NB], F32, tag="m")
                step = small.tile([P, NB], F32, tag="step")
                rstep = small.tile([P, NB], F32, tag="rstep")
                y8 = i8p.tile([P, NB, BLK], I8)
                q = qout.tile([P, C], F32)
                q3 = q.rearrange("p (b k) -> p b k", k=BLK)

                # boundary tiles run the whole pipeline per column-chunk so
                # the pipeline ramp (first tile: DVE starts after one small
                # load) and tail (last tile: trailing store is one small
                # chunk) shrink from ~13us to ~3-5us; interior tiles run
                # full-width (chunking them only adds instruction overhead)
                if i == 0:
                    widths = [512, 2560, 2560, 2560]
                elif i == N_TILES - 1:
                    widths = [4096, 2048, 1536, 512]
                else:
                    widths = [C]
                c0 = 0
                for ci, cw in enumerate(widths):
                    bs = slice(c0 // BLK, (c0 + cw) // BLK)
                    bw = cw // BLK
                    nc.sync.dma_start(
                        out=x_t[:, c0 : c0 + cw], in_=x_ext[rows, c0 : c0 + cw]
                    )
                    nc.vector.tensor_reduce(
                        out=m[:, bs],
                        in_=x3[:, bs, :],
                        axis=mybir.AxisListType.X,
                        op=mybir.AluOpType.max,
                        apply_absolute_value=True,
                    )
                    nc.vector.tensor_scalar(
                        out=step[:, bs].bitcast(I32),
                        in0=m[:, bs].bitcast(I32),
                        scalar1=0x7F800000,
                        scalar2=None,
                        op0=mybir.AluOpType.bitwise_and,
                    )
                    nc.vector.tensor_scalar(
                        out=step[:, bs].bitcast(I32),
                        in0=step[:, bs].bitcast(I32),
                        scalar1=0x03800000,
                        scalar2=0x03000000,
                        op0=mybir.AluOpType.max,
                        op1=mybir.AluOpType.subtract,
                    )
                    # rstep_bits = 0x7F000000 - step_bits via reverse-subtract
                    # (single-src tensor_scalar runs in the DVE 2x mode; the
                    # reverse0 BIR field is not exposed by the python wrapper)
                    rs_inst = nc.vector.tensor_scalar(
                        out=rstep[:, bs].bitcast(I32),
                        in0=step[:, bs].bitcast(I32),
                        scalar1=0x7F000000,
                        scalar2=None,
                        op0=mybir.AluOpType.subtract,
                    )
                    (rs_inst.ins if hasattr(rs_inst, "ins") else rs_inst).reverse0 = True
                    nc.vector.tensor_tensor(
                        out=y8[:, bs, :],
                        in0=x3[:, bs, :],
                        in1=rstep[:, bs].unsqueeze(2).broadcast_to((P, bw, BLK)),
                        op=mybir.AluOpType.mult,
                    )
                    nc.vector.scalar_tensor_tensor(
                        out=q3[:, bs, :],
                        in0=y8[:, bs, :],
                        scalar=-127.0,
                        in1=step[:, bs].unsqueeze(2).broadcast_to((P, bw, BLK)),
                        op0=mybir.AluOpType.max,
                        op1=mybir.AluOpType.mult,
                    )
                    eng = nc.scalar if ci % 2 == 0 else nc.sync
                    eng.dma_start(
                        out=out_ext[rows, c0 : c0 + cw], in_=q[:, c0 : c0 + cw]
                    )
                    c0 += cw
    return nc


_cached_nc = None


def run(x: np.ndarray, trace: bool = False):
    """Run the SPMD kernel on 8 cores; returns (full_output, BassKernelResults)."""
    global _cached_nc
    _apply_bir_fix()
    from concourse.bass_utils import run_bass_kernel_spmd

    assert x.shape == (R_FULL, C) and x.dtype == np.float32
    if _cached_nc is None:
        _cached_nc = build_program()

    in_maps = [
        {"x": np.ascontiguousarray(x[i * R_LOCAL : (i + 1) * R_LOCAL])}
        for i in range(N_CORES)
    ]
    res = run_bass_kernel_spmd(
        _cached_nc, in_maps, list(range(N_CORES)), trace=trace
    )
    out = np.concatenate([r["out"] for r in res.results], axis=0)
    return out, res


def kernel(x: np.ndarray) -> np.ndarray:
    out, _ = run(x, trace=False)
    return out



# revision 3
# speedup vs baseline: 1.0211x; 1.0211x over previous
"""BFP (block floating point) quantizer for Trainium2, 8 NeuronCores.

Reference semantics (BITWIDTH=16, BLOCK_SIZE=16, AXIS=1):
  per 16-element block along axis 1:
    max_abs = max |x|                     (block reduction)
    shared_exp = frexp(max_abs).e - 1
    step = 2^(shared_exp - 6)
    q = clip(round_half_even(x / step), -127, 127) * step
    q = 0 where max_abs == 0

Kernel mapping (per [128, 8192] f32 tile, blocks of 16 on the free axis):
  1. m = tensor_reduce(max, abs) over [128, 512, 16]        -> block max-abs
  2. exponent bit tricks on the int32 view of m:
       masked     = m_bits & 0x7F800000          (sign is 0, m >= 0)
       step_bits  = max(masked, 7<<23) - 6<<23   (power of two, exact;
                                                  the max() guards all-zero /
                                                  denormal blocks)
       rstep_bits = 0x7F000000 - step_bits       (exact reciprocal: exponents
                                                  sum to 254)
  3. y_i8 = tensor_tensor(x, rstep_bcast, mult) with int8 output.
     x * rstep is exact in fp32 (power-of-two scale); the DVE output
     converter does RNE + saturation, which implements round-half-even and
     the upper clip in one pass (verified bit-exact on HW).  The only
     deviation from the reference is y in (-128, -127.5] -> -128 (reference
     clips to -127), fixed in step 4.
  4. q = scalar_tensor_tensor(y_i8, -127.0, step_bcast, max, mult) -> f32.

Sharding: trivially data-parallel on axis 0; each of the 8 cores gets a
[1024, 8192] row shard and runs 8 [128, 8192] tiles.

Performance notes (measured via in-NEFF reps slope on this container's HW):
  - pure-DMA floor (loads+stores only): ~200.6 us; read-only runs at
    ~402 GB/s and write-only at ~379 GB/s, so the mixed-direction rate
    (~334 GB/s) is an SDMA read/write-interleave property, not a queue
    artifact (single-ring FIFO issue measured the same).
  - compute-only (resident tiles): ~215 us; per-op isolated times sum to
    ~174 us.  The difference is ~0.4 us/instruction of DVE dispatch/sem
    overhead (the tile scheduler self-serializes DVE via a semaphore chain),
    so fewer+bigger instructions win over finer chunking.
  - GpSimd offload (dequant TT / split) regresses: Q7 streaming is ~3x
    slower than the cost model and contends with DVE for the shared SBUF
    port slot.  Act engine cannot apply per-block scales (per-partition
    only).  A 2x_1p 16-bit dequant never engages (mixed-dtype TT runs 1x).
  - xin bufs=3 (deeper load-ahead) beats bufs=2 by ~4% in drift-cancelling
    A/B; boundary tiles keep the ramped column splits so the first store
    issues ~4 us into the pass and the tail store is small.
"""

import sys

for _p in ("/opt/trn_rl_repo",):
    if _p not in sys.path:
        sys.path.append(_p)

import json

import numpy as np

N_CORES = 8
R_FULL = 8192
C = 8192
R_LOCAL = R_FULL // N_CORES  # 1024
P = 128
BLK = 16
NB = C // BLK  # 512
N_TILES = R_LOCAL // P  # 8


# ---------------------------------------------------------------------------
# Workaround for this container's walrus build: it encodes at most ONE
# semaphore wait per instruction ("Too many sync wait commands").  First merge
# sem-ge-imm waits on the same semaphore (keep the max target), then rewrite
# the serialized BIR so any instruction with N>1 remaining waits is preceded
# by N-1 same-engine NoOps carrying one wait each.
# ---------------------------------------------------------------------------
def _merge_waits(waits):
    merged = []
    best = {}
    for w in waits:
        if w.get("sync_type") == "semaphore" and w.get("wait_mode") == "sem-ge-imm":
            key = (w.get("id"), w.get("ant_name"))
            if key in best:
                if w.get("wait_value", 0) > best[key].get("wait_value", 0):
                    best[key]["wait_value"] = w["wait_value"]
                continue
            best[key] = w = dict(w)
        merged.append(w)
    return merged


def _split_multiwaits(bir_json: bytes) -> bytes:
    j = json.loads(bir_json)
    ctr = 0
    changed = False
    for fn in j.get("functions", []):
        for bb in fn.get("blocks", []):
            new_insts = []
            for ins in bb.get("instructions", []):
                si = ins.get("sync_info")
                waits = (si or {}).get("on_wait") or []
                if len(waits) > 1:
                    mw = _merge_waits(waits)
                    if len(mw) != len(waits):
                        changed = True
                        si["on_wait"] = waits = mw
                if len(waits) > 1:
                    changed = True
                    for w in waits[:-1]:
                        ctr += 1
                        carrier = {
                            "engine": ins["engine"],
                            "ins": [],
                            "outs": [],
                            "name": f"WSPLIT-{ctr}",
                            "opcode": "NoOp",
                            "text_hint": "wait_split",
                            "sync_info": {"on_wait": [w], "on_update": []},
                        }
                        if "debug" in ins:
                            carrier["debug"] = ins["debug"]
                        new_insts.append(carrier)
                    si["on_wait"] = [waits[-1]]
                new_insts.append(ins)
            bb["instructions"] = new_insts
    if not changed:
        return bir_json
    return json.dumps(j).encode()


_hook_applied = False


def _apply_bir_fix():
    global _hook_applied
    if _hook_applied:
        return
    _hook_applied = True
    from concourse import bass2jax

    orig = bass2jax.compile_bir_kernel

    def wrapper(bir_json, tmpdir, neff_name="file.neff"):
        return orig(_split_multiwaits(bytes(bir_json)), tmpdir, neff_name)

    bass2jax.compile_bir_kernel = wrapper


# ---------------------------------------------------------------------------
# Program construction
# ---------------------------------------------------------------------------
def build_program(reps: int = 1):
    """reps>1 wraps the whole tile loop in a dynamic For_i — used only for
    benchmarking (amortizes the ~80ms axon dispatch overhead)."""
    from contextlib import nullcontext

    import concourse.bass as bass
    import concourse.tile as tile
    from concourse import mybir

    F32 = mybir.dt.float32
    I32 = mybir.dt.int32
    I8 = mybir.dt.int8

    nc = bass.Bass("TRN2", target_bir_lowering=False)
    x_ext = nc.dram_tensor("x", [R_LOCAL, C], F32, kind="ExternalInput")
    out_ext = nc.dram_tensor("out", [R_LOCAL, C], F32, kind="ExternalOutput")

    with tile.TileContext(nc) as tc:
        with (
            tc.tile_pool(name="xin", bufs=3) as xin,
            tc.tile_pool(name="qout", bufs=2) as qout,
            tc.tile_pool(name="i8p", bufs=2) as i8p,
            tc.tile_pool(name="small", bufs=2) as small,
            tc.For_i(0, reps, 1) if reps > 1 else nullcontext(),
        ):
            for i in range(N_TILES):
                rows = slice(i * P, (i + 1) * P)

                x_t = xin.tile([P, C], F32)
                x3 = x_t.rearrange("p (b k) -> p b k", k=BLK)
                m = small.tile([P, NB], F32, tag="m")
                step = small.tile([P, NB], F32, tag="step")
                rstep = small.tile([P, NB], F32, tag="rstep")
                y8 = i8p.tile([P, NB, BLK], I8)
                q = qout.tile([P, C], F32)
                q3 = q.rearrange("p (b k) -> p b k", k=BLK)

                # boundary tiles run the whole pipeline per column-chunk so
                # the pipeline ramp (first tile: DVE starts after one small
                # load) and tail (last tile: trailing store is one small
                # chunk) shrink from ~13us to ~3-5us; interior tiles run
                # full-width (chunking them only adds instruction overhead)
                if i == 0:
                    widths = [512, 2560, 2560, 2560]
                elif i == N_TILES - 1:
                    widths = [4096, 2048, 1536, 512]
                else:
                    widths = [C]
                c0 = 0
                for cw in widths:
                    bs = slice(c0 // BLK, (c0 + cw) // BLK)
                    bw = cw // BLK
                    nc.sync.dma_start(
                        out=x_t[:, c0 : c0 + cw], in_=x_ext[rows, c0 : c0 + cw]
                    )
                    nc.vector.tensor_reduce(
                        out=m[:, bs],
                        in_=x3[:, bs, :],
                        axis=mybir.AxisListType.X,
                        op=mybir.AluOpType.max,
                        apply_absolute_value=True,
                    )
                    nc.vector.tensor_scalar(
                        out=step[:, bs].bitcast(I32),
                        in0=m[:, bs].bitcast(I32),
                        scalar1=0x7F800000,
                        scalar2=None,
                        op0=mybir.AluOpType.bitwise_and,
                    )
                    nc.vector.tensor_scalar(
                        out=step[:, bs].bitcast(I32),
                        in0=step[:, bs].bitcast(I32),
                        scalar1=0x03800000,
                        scalar2=0x03000000,
                        op0=mybir.AluOpType.max,
                        op1=mybir.AluOpType.subtract,
                    )
                    # rstep_bits = 0x7F000000 - step_bits via reverse-subtract
                    # (single-src tensor_scalar runs in the DVE 2x mode; the
                    # reverse0 BIR field is not exposed by the python wrapper)
                    rs_inst = nc.vector.tensor_scalar(
                        out=rstep[:, bs].bitcast(I32),
                        in0=step[:, bs].bitcast(I32),
                        scalar1=0x7F000000,
                        scalar2=None,
                        op0=mybir.AluOpType.subtract,
                    )
                    (rs_inst.ins if hasattr(rs_inst, "ins") else rs_inst).reverse0 = True
                    nc.vector.tensor_tensor(
                        out=y8[:, bs, :],
                        in0=x3[:, bs, :],
                        in1=rstep[:, bs].unsqueeze(2).broadcast_to((P, bw, BLK)),
                        op=mybir.AluOpType.mult,
                    )
                    nc.vector.scalar_tensor_tensor(
                        out=q3[:, bs, :],
                        in0=y8[:, bs, :],
                        scalar=-127.0,
                        in1=step[:, bs].unsqueeze(2).broadcast_to((P, bw, BLK)),
                        op0=mybir.AluOpType.max,
                        op1=mybir.AluOpType.mult,
                    )
                    nc.scalar.dma_start(
                        out=out_ext[rows, c0 : c0 + cw], in_=q[:, c0 : c0 + cw]
                    )
                    c0 += cw
    return nc


_cached_nc = None


def run(x: np.ndarray, trace: bool = False):
    """Run the SPMD kernel on 8 cores; returns (full_output, BassKernelResults)."""
    global _cached_nc
    _apply_bir_fix()
    from concourse.bass_utils import run_bass_kernel_spmd

    assert x.shape == (R_FULL, C) and x.dtype == np.float32
    if _cached_nc is None:
        _cached_nc = build_program()

    in_maps = [
        {"x": np.ascontiguousarray(x[i * R_LOCAL : (i + 1) * R_LOCAL])}
        for i in range(N_CORES)
    ]
    res = run_bass_kernel_spmd(
        _cached_nc, in_maps, list(range(N_CORES)), trace=trace
    )
    out = np.concatenate([r["out"] for r in res.results], axis=0)
    return out, res


def kernel(x: np.ndarray) -> np.ndarray:
    out, _ = run(x, trace=False)
    return out
